# revision 22
# baseline (speedup 1.0000x reference)
"""AnomalyAttention TRN2 kernel: B=8 batch-sharded across 8 NeuronCores.

Per core (one batch element):
  - scores = Q K^T (bf16 matmuls, contraction over E=64 via host-pretransposed
    Q^T/K^T layouts), causal mask added via an identity-matmul of a -1e30
    upper-triangular constant into the diagonal 128-block.
  - series = softmax(scale*scores) row-wise: ACT Exp with accum_out giving the
    row sums, DVE reciprocal + per-partition tensor_scalar multiply.
  - A^T tiles produced by PE transposes of the normalized series (bf16), then
    V_out^T = V^T A^T accumulated on PE, re-transposed to [l, d].
  - prior^T[s,i] = sum_h exp(-|i-s| / sigma[h,s]) computed with ACT Exp using
    per-partition scale = 1/sigma (reciprocal on device); only the tridiagonal
    128-tile band is computed (sigma<=2 => exp==0 beyond |i-s|>=129), head-sum
    via a DVE/GPSIMD add tree, transposed back to [i,s] on PE, normalized by
    per-partition reciprocal of the row sums (tensor_scalar accum_out).
Outputs are written bf16 and upcast to f32 on the host; the strictly-dead
(causally masked / out-of-band) regions are exact zeros filled on the host.
"""

import os
import numpy as np
import ml_dtypes

B, L, H, E = 8, 1024, 8, 64
S, D = L, E
NCORES = 8
NEG = -1.0e30
SCALE = 1.0 / float(E) ** 0.5
BF16 = ml_dtypes.bfloat16


def _build_graph():
    import concourse.bass as bass
    import concourse.tile as tile
    from concourse import bacc, mybir

    f32 = mybir.dt.float32
    bf16 = mybir.dt.bfloat16
    Exp = mybir.ActivationFunctionType.Exp
    mult = mybir.AluOpType.mult
    add = mybir.AluOpType.add

    nc = bacc.Bacc(
        "TRN2",
        target_bir_lowering=False,
        debug=False,
        enable_asserts=False,
        num_devices=NCORES,
    )

    qt_d = nc.dram_tensor("qt", [H, E, L], bf16, kind="ExternalInput")
    kt_d = nc.dram_tensor("kt", [H, E, L], bf16, kind="ExternalInput")
    v_d = nc.dram_tensor("v", [S, H * D], bf16, kind="ExternalInput")
    sig_d = nc.dram_tensor("sigt", [S, H], f32, kind="ExternalInput")
    dist_d = nc.dram_tensor("ndist", [S, L], bf16, kind="ExternalInput")
    idb_d = nc.dram_tensor("idb", [128, 128], bf16, kind="ExternalInput")
    mask_d = nc.dram_tensor("maskb", [128, 128], bf16, kind="ExternalInput")

    ser_d = nc.dram_tensor("series", [H, L, S], bf16, kind="ExternalOutput")
    pri_d = nc.dram_tensor("prior", [L, S], bf16, kind="ExternalOutput")
    vo_d = nc.dram_tensor("vout", [L, H * D], bf16, kind="ExternalOutput")

    NT = L // 128  # 8 l/s tiles

    with tile.TileContext(nc) as tc:
        with (
            tc.tile_pool(name="resident", bufs=1) as res,
            tc.tile_pool(name="pex", bufs=16) as pex_p,
            tc.tile_pool(name="tree", bufs=6) as tree_p,
            tc.tile_pool(name="esb", bufs=10) as esb_p,
            tc.tile_pool(name="ssb", bufs=4) as ssb_p,
            tc.tile_pool(name="cols", bufs=8) as col_p,
            tc.tile_pool(name="prow", bufs=2) as prow_p,
            tc.tile_pool(name="vts", bufs=2) as vts_p,
            tc.tile_pool(name="psc", bufs=2, space="PSUM") as psc_p,
            tc.tile_pool(name="pat", bufs=2, space="PSUM") as pat_p,
            tc.tile_pool(name="pmix", bufs=2, space="PSUM") as pmix_p,
        ):
            # ---- resident allocations ----
            qt_sb = res.tile([E, H, L], bf16, tag="qt_sb")
            kt_sb = res.tile([E, H, L], bf16, tag="kt_sb")
            v_sb = res.tile([128, NT, H * D], bf16, tag="v_sb")
            dist_sb = res.tile([128, NT, L], bf16, tag="dist_sb")
            idb_sb = res.tile([128, 128], bf16, tag="idb_sb")
            mask_sb = res.tile([128, 128], bf16, tag="mask_sb")
            invsig_sb = res.tile([128, NT, H], f32, tag="invsig_sb")
            sig_sb = res.tile([128, NT, H], f32, tag="sig_sb")
            aT_sb = res.tile([128, NT, L], bf16, tag="aT_sb")
            psT_sb = res.tile([128, NT, L], bf16, tag="psT_sb")
            vout_sb = res.tile([128, NT, H * D], bf16, tag="vout_sb")

            # j stored at slot: evens first then odds (bank-alternating copies)
            def jslot(j):
                return j // 2 if j % 2 == 0 else 4 + j // 2

            # ---- critical first loads; the rest staged into early heads ----
            nc.sync.dma_start(qt_sb[:, 0, :], qt_d[0])
            nc.sync.dma_start(kt_sb[:, 0, :], kt_d[0])
            nc.sync.dma_start(idb_sb[:], idb_d[:])
            nc.sync.dma_start(mask_sb[:], mask_d[:])
            warm = col_p.tile([128, 128], bf16, tag="warm")
            nc.scalar.activation(warm[:], idb_sb[:], Exp, scale=1.0)
            nc.sync.dma_start(dist_sb[:, 0, :], dist_d[0:128, :])
            nc.sync.dma_start(sig_sb[:, 0, :], sig_d[0:128, :])
            nc.vector.reciprocal(invsig_sb[:, 0, :], sig_sb[:, 0, :])
            # zero the dead (l < 128*j) region of each A^T slot
            for j in range(1, NT):
                nc.vector.memset(aT_sb[:, j, 0:128 * j], 0.0)
            # zero the psT band edges the 256-wide prior exp won't write
            for j in range(NT):
                blo = max(0, 128 * (j - 1))
                bhi = min(L, 128 * (j + 2))
                lo = max(0, 128 * j - 32)
                hi = min(L, 128 * (j + 1) + 32)
                if blo < lo:
                    nc.vector.memset(psT_sb[:, j, blo:lo], 0.0)
                if hi < bhi:
                    nc.vector.memset(psT_sb[:, j, hi:bhi], 0.0)

            def load_block(h):
                # staged: before head h runs, load head h+1 inputs and the
                # prior j=h(+1) inputs; v tiles arrive before AV of head 0
                if h + 1 < H:
                    nc.sync.dma_start(qt_sb[:, h + 1, :], qt_d[h + 1])
                    nc.sync.dma_start(kt_sb[:, h + 1, :], kt_d[h + 1])
                if h + 1 < NT:
                    j = h + 1
                    nc.sync.dma_start(dist_sb[:, j, :], dist_d[128 * j:128 * (j + 1), :])
                    nc.sync.dma_start(sig_sb[:, j, :], sig_d[128 * j:128 * (j + 1), :])
                    nc.vector.reciprocal(invsig_sb[:, j, :], sig_sb[:, j, :])
                if h == 0:
                    for j in range(NT):
                        nc.sync.dma_start(v_sb[:, j, :], v_d[128 * j:128 * (j + 1), :])

            def prior_exp_block(j):
                lo = max(0, 128 * j - 32)
                hi = min(L, 128 * (j + 1) + 32)
                Wj = hi - lo
                pex = []
                for h in range(H):
                    t = pex_p.tile([128, 256], bf16, tag="pex")
                    nc.scalar.activation(
                        t[:, 0:Wj], dist_sb[:, j, lo:hi], Exp,
                        scale=invsig_sb[:, j, h:h + 1],
                    )
                    pex.append(t)
                s01 = tree_p.tile([128, 256], bf16, tag="tr")
                s23 = tree_p.tile([128, 256], bf16, tag="tr")
                s45 = tree_p.tile([128, 256], bf16, tag="tr")
                s67 = tree_p.tile([128, 256], bf16, tag="tr")
                nc.gpsimd.tensor_tensor(s01[:, 0:Wj], pex[0][:, 0:Wj], pex[1][:, 0:Wj], add)
                nc.gpsimd.tensor_tensor(s23[:, 0:Wj], pex[2][:, 0:Wj], pex[3][:, 0:Wj], add)
                nc.vector.tensor_tensor(s45[:, 0:Wj], pex[4][:, 0:Wj], pex[5][:, 0:Wj], add)
                nc.vector.tensor_tensor(s67[:, 0:Wj], pex[6][:, 0:Wj], pex[7][:, 0:Wj], add)
                s0123 = tree_p.tile([128, 256], bf16, tag="tr2")
                s4567 = tree_p.tile([128, 256], bf16, tag="tr2")
                nc.vector.tensor_tensor(s0123[:, 0:Wj], s01[:, 0:Wj], s23[:, 0:Wj], add)
                nc.vector.tensor_tensor(s4567[:, 0:Wj], s45[:, 0:Wj], s67[:, 0:Wj], add)
                nc.vector.tensor_tensor(psT_sb[:, j, lo:hi], s0123[:, 0:Wj], s4567[:, 0:Wj], add)

            def prior_out_block(c):
                jlo = max(0, c - 1)
                jhi = min(NT, c + 2)
                nb = jhi - jlo
                Ws = 128 * nb
                pT = pmix_p.tile([128, 3, 128], bf16, tag="pmix")
                for jj, j in enumerate(range(jlo, jhi)):
                    nc.tensor.transpose(pT[:, jj, :],
                                        psT_sb[:, j, 128 * c:128 * (c + 1)],
                                        idb_sb[:])
                denp = col_p.tile([128, 1], f32, tag="col")
                nc.vector.tensor_reduce(denp[:], pT[:, 0:nb, :],
                                        axis=mybir.AxisListType.XYZW,
                                        op=add)
                rp = col_p.tile([128, 1], f32, tag="col")
                nc.vector.reciprocal(rp[:], denp[:])
                rows3 = prow_p.tile([128, 3, 128], bf16, tag="prow3")
                nc.vector.tensor_scalar_mul(rows3[:, 0:nb, :], pT[:, 0:nb, :], rp[:])
                nc.sync.dma_start(
                    pri_d[128 * c:128 * (c + 1), 128 * jlo:128 * jhi],
                    rows3[:, 0:nb, :],
                )

            def head_block(h):
                denh = col_p.tile([128, NT], f32, tag="denh")
                rch = col_p.tile([128, NT], f32, tag="denh")
                esbs = []
                for i in range(NT):
                    pre = 128 * i
                    W = pre + 128
                    psc = psc_p.tile([128, 1024], f32, tag="psc")
                    qsl = qt_sb[:, h, pre:pre + 128]
                    nch = (W + 511) // 512
                    for cc in range(nch):
                        e0 = 512 * cc
                        e1 = min(W, 512 * (cc + 1))
                        last = cc == nch - 1
                        nc.tensor.matmul(
                            psc[:, e0:e1], qsl, kt_sb[:, h, e0:e1],
                            start=True, stop=not last,
                            skip_group_check=True,
                        )
                    nc.tensor.matmul(psc[:, pre:W], idb_sb[:], mask_sb[:],
                                     start=False, stop=True,
                                     skip_group_check=True)
                    esb = esb_p.tile([128, 1024], bf16, tag="esb")
                    nc.scalar.activation(esb[:, 0:W], psc[:, 0:W], Exp,
                                         scale=SCALE, accum_out=denh[:, i:i + 1])
                    esbs.append(esb)
                nc.vector.reciprocal(rch[:], denh[:])
                vT = vts_p.tile([64, L], bf16, tag="vT")
                for i in range(NT):
                    pre = 128 * i
                    W = pre + 128
                    esb = esbs[i]
                    ssb = ssb_p.tile([128, 1024], bf16, tag="ssb")
                    nc.vector.tensor_scalar_mul(ssb[:, 0:W], esb[:, 0:W],
                                                rch[:, i:i + 1])
                    nc.sync.dma_start(ser_d[h, pre:pre + 128, 0:W], ssb[:, 0:W])
                    at = pat_p.tile([128, 8, 128], bf16, tag="paT")
                    for j in range(i + 1):
                        nc.tensor.transpose(at[:, j, :],
                                            ssb[:, 128 * j:128 * (j + 1)],
                                            idb_sb[:])
                    nc.vector.tensor_copy(aT_sb[:, 0:i + 1, pre:pre + 128],
                                          at[:, 0:i + 1, :])
                for cch in range(2):
                    pv = pmix_p.tile([64, 512], f32, tag="pmix")
                    njv = 4 * (cch + 1)
                    for j in range(njv):
                        nc.tensor.matmul(
                            pv[:], v_sb[:, j, D * h:D * (h + 1)],
                            aT_sb[:, j, 512 * cch:512 * (cch + 1)],
                            start=(j == 0), stop=(j == njv - 1),
                        )
                    nc.vector.tensor_copy(vT[:, 512 * cch:512 * (cch + 1)], pv[:])
                prE = pmix_p.tile([128, 4, 64], bf16, tag="pmix")
                prO = pmix_p.tile([128, 4, 64], bf16, tag="pmix")
                for t in range(NT):
                    dst = prE if t % 2 == 0 else prO
                    nc.tensor.transpose(dst[:, t // 2, :],
                                        vT[:, 128 * t:128 * (t + 1)],
                                        idb_sb[0:64, 0:64])
                nc.vector.tensor_copy(vout_sb[:, 0:4, D * h:D * (h + 1)], prE[:])
                nc.vector.tensor_copy(vout_sb[:, 4:8, D * h:D * (h + 1)], prO[:])

            for h in range(H):
                load_block(h)
                prior_exp_block(h)
                if h >= 1:
                    prior_out_block(h - 1)
                head_block(h)
            prior_out_block(NT - 1)

            for t in range(NT):
                tsl = t // 2 if t % 2 == 0 else 4 + t // 2
                nc.sync.dma_start(vo_d[128 * t:128 * (t + 1), :],
                                  vout_sb[:, tsl, :])

    nc.finalize()
    return nc


_NC_CACHE = {}


def _get_nc():
    if "nc" not in _NC_CACHE:
        _NC_CACHE["nc"] = _build_graph()
    return _NC_CACHE["nc"]


def kernel(queries, keys, values, sigma):
    from concourse.bass_utils import run_bass_kernel_spmd

    assert queries.shape == (B, L, H, E)
    nc = _get_nc()

    idb = np.eye(128, dtype=np.float32).astype(BF16)
    maskb = np.triu(np.full((128, 128), NEG, dtype=np.float32), k=1).astype(BF16)
    ndist = -np.abs(
        np.arange(L, dtype=np.float32)[:, None] - np.arange(S, dtype=np.float32)[None, :]
    ).astype(np.float32)
    ndist_b = ndist.astype(BF16)

    in_maps = []
    for b in range(NCORES):
        in_maps.append({
            "qt": np.ascontiguousarray(queries[b].astype(BF16).transpose(1, 2, 0)),
            "kt": np.ascontiguousarray(keys[b].astype(BF16).transpose(1, 2, 0)),
            "v": np.ascontiguousarray(values[b].astype(BF16).reshape(S, H * D)),
            "sigt": np.ascontiguousarray(sigma[b].T.astype(np.float32)),
            "ndist": ndist_b,
            "idb": idb,
            "maskb": maskb,
        })

    trace = bool(int(os.environ.get("ANOM_TRACE", "0")))
    res = run_bass_kernel_spmd(nc, in_maps, core_ids=list(range(NCORES)), trace=trace)
    if trace and res.exec_time_ns is not None:
        print(f"HW exec time: {res.exec_time_ns} ns")
        if res.instructions_and_trace is not None:
            print("trace:", res.instructions_and_trace[1])

    Vout = np.zeros((B, L, H, D), dtype=np.float32)
    series = np.zeros((B, H, L, S), dtype=np.float32)
    prior = np.zeros((B, L, S), dtype=np.float32)
    for b in range(NCORES):
        r = res.results[b]
        Vout[b] = r["vout"].astype(np.float32).reshape(L, H, D)
        sb = r["series"].astype(np.float32)
        for i in range(L // 128):
            sb[:, 128 * i:128 * (i + 1), 128 * (i + 1):] = 0.0
        series[b] = sb
        pb = r["prior"].astype(np.float32)
        for c in range(L // 128):
            jlo, jhi = max(0, 128 * (c - 1)), min(S, 128 * (c + 2))
            prior[b, 128 * c:128 * (c + 1), :jlo] = 0.0
            prior[b, 128 * c:128 * (c + 1), jhi:] = 0.0
            prior[b, 128 * c:128 * (c + 1), jlo:jhi] = pb[128 * c:128 * (c + 1), jlo:jhi]
    return Vout, series, prior


# revision 23
# speedup vs baseline: 1.2115x; 1.2115x over previous
"""AnomalyAttention TRN2 kernel: B=8 batch-sharded across 8 NeuronCores.

Per core (one batch element):
  - scores = Q K^T (bf16 matmuls, contraction over E=64 via host-pretransposed
    Q^T/K^T layouts), causal mask added via an identity-matmul of a -1e30
    upper-triangular constant into the diagonal 128-block.
  - series = softmax(scale*scores) row-wise: ACT Exp with accum_out giving the
    row sums, DVE reciprocal + per-partition tensor_scalar multiply.
  - A^T tiles produced by PE transposes of the normalized series (bf16), then
    V_out^T = V^T A^T accumulated on PE, re-transposed to [l, d].
  - prior^T[s,i] = sum_h exp(-|i-s| / sigma[h,s]) computed with ACT Exp using
    per-partition scale = 1/sigma (reciprocal on device); only the tridiagonal
    128-tile band is computed (sigma<=2 => exp==0 beyond |i-s|>=129), head-sum
    via a DVE/GPSIMD add tree, transposed back to [i,s] on PE, normalized by
    per-partition reciprocal of the row sums (tensor_scalar accum_out).
Outputs are written bf16 and upcast to f32 on the host; the strictly-dead
(causally masked / out-of-band) regions are exact zeros filled on the host.
"""

import os
import numpy as np
import ml_dtypes

B, L, H, E = 8, 1024, 8, 64
S, D = L, E
NCORES = 8
NEG = -1.0e30
SCALE = 1.0 / float(E) ** 0.5
BF16 = ml_dtypes.bfloat16


def _build_graph():
    import concourse.bass as bass
    import concourse.tile as tile
    from concourse import bacc, mybir

    f32 = mybir.dt.float32
    bf16 = mybir.dt.bfloat16
    Exp = mybir.ActivationFunctionType.Exp
    mult = mybir.AluOpType.mult
    add = mybir.AluOpType.add

    nc = bacc.Bacc(
        "TRN2",
        target_bir_lowering=False,
        debug=False,
        enable_asserts=False,
        num_devices=NCORES,
    )

    qt_d = nc.dram_tensor("qt", [H, E, L], bf16, kind="ExternalInput")
    kt_d = nc.dram_tensor("kt", [H, E, L], bf16, kind="ExternalInput")
    v_d = nc.dram_tensor("v", [S, H * D], bf16, kind="ExternalInput")
    sig_d = nc.dram_tensor("sigt", [S, H], f32, kind="ExternalInput")
    dist_d = nc.dram_tensor("ndist", [S, L], bf16, kind="ExternalInput")
    idb_d = nc.dram_tensor("idb", [128, 128], bf16, kind="ExternalInput")
    mask_d = nc.dram_tensor("maskb", [128, 128], bf16, kind="ExternalInput")

    ser_d = nc.dram_tensor("series", [H, L, S], bf16, kind="ExternalOutput")
    pri_d = nc.dram_tensor("prior", [L, S], bf16, kind="ExternalOutput")
    vo_d = nc.dram_tensor("vout", [L, H * D], bf16, kind="ExternalOutput")

    NT = L // 128  # 8 l/s tiles

    with tile.TileContext(nc) as tc:
        with (
            tc.tile_pool(name="resident", bufs=1) as res,
            tc.tile_pool(name="pex", bufs=16) as pex_p,
            tc.tile_pool(name="tree", bufs=6) as tree_p,
            tc.tile_pool(name="esb", bufs=10) as esb_p,
            tc.tile_pool(name="ssb", bufs=6) as ssb_p,
            tc.tile_pool(name="cols", bufs=8) as col_p,
            tc.tile_pool(name="prow", bufs=2) as prow_p,
            tc.tile_pool(name="vts", bufs=2) as vts_p,
            tc.tile_pool(name="psc", bufs=2, space="PSUM") as psc_p,
            tc.tile_pool(name="pat", bufs=2, space="PSUM") as pat_p,
            tc.tile_pool(name="pmix", bufs=2, space="PSUM") as pmix_p,
        ):
            # ---- resident allocations ----
            qt_sb = res.tile([E, H, L], bf16, tag="qt_sb")
            kt_sb = res.tile([E, H, L], bf16, tag="kt_sb")
            v_sb = res.tile([128, NT, H * D], bf16, tag="v_sb")
            dist_sb = res.tile([128, NT, L], bf16, tag="dist_sb")
            idb_sb = res.tile([128, 128], bf16, tag="idb_sb")
            mask_sb = res.tile([128, 128], bf16, tag="mask_sb")
            invsig_sb = res.tile([128, NT, H], f32, tag="invsig_sb")
            sig_sb = res.tile([128, NT, H], f32, tag="sig_sb")
            aT_sb = res.tile([128, NT, L], bf16, tag="aT_sb")
            psT_sb = res.tile([128, NT, L], bf16, tag="psT_sb")
            vout_sb = res.tile([128, NT, H * D], bf16, tag="vout_sb")

            # j stored at slot: evens first then odds (bank-alternating copies)
            def jslot(j):
                return j // 2 if j % 2 == 0 else 4 + j // 2

            # ---- critical first loads; the rest staged into early heads ----
            nc.sync.dma_start(qt_sb[:, 0, :], qt_d[0])
            nc.sync.dma_start(kt_sb[:, 0, :], kt_d[0])
            nc.sync.dma_start(idb_sb[:], idb_d[:])
            nc.sync.dma_start(mask_sb[:], mask_d[:])
            warm = col_p.tile([128, 128], bf16, tag="warm")
            nc.scalar.activation(warm[:], idb_sb[:], Exp, scale=1.0)
            nc.sync.dma_start(dist_sb[:, 0, :], dist_d[0:128, :])
            nc.sync.dma_start(sig_sb[:, 0, :], sig_d[0:128, :])
            nc.vector.reciprocal(invsig_sb[:, 0, :], sig_sb[:, 0, :])
            # zero the dead (l < 128*j) region of each A^T slot
            for j in range(1, NT):
                nc.vector.memset(aT_sb[:, j, 0:128 * j], 0.0)
            # zero the psT band edges the 256-wide prior exp won't write
            for j in range(NT):
                blo = max(0, 128 * (j - 1))
                bhi = min(L, 128 * (j + 2))
                lo = max(0, 128 * j - 32)
                hi = min(L, 128 * (j + 1) + 32)
                if blo < lo:
                    nc.vector.memset(psT_sb[:, j, blo:lo], 0.0)
                if hi < bhi:
                    nc.vector.memset(psT_sb[:, j, hi:bhi], 0.0)

            def load_block(h):
                # staged: before head h runs, load head h+1 inputs and the
                # prior j=h(+1) inputs; v tiles arrive before AV of head 0
                if h + 1 < H:
                    nc.sync.dma_start(qt_sb[:, h + 1, :], qt_d[h + 1])
                    nc.sync.dma_start(kt_sb[:, h + 1, :], kt_d[h + 1])
                if h + 1 < NT:
                    j = h + 1
                    nc.sync.dma_start(dist_sb[:, j, :], dist_d[128 * j:128 * (j + 1), :])
                    nc.sync.dma_start(sig_sb[:, j, :], sig_d[128 * j:128 * (j + 1), :])
                    nc.vector.reciprocal(invsig_sb[:, j, :], sig_sb[:, j, :])
                if h == 0:
                    for j in range(NT):
                        nc.sync.dma_start(v_sb[:, j, :], v_d[128 * j:128 * (j + 1), :])

            def prior_exp_block(j):
                lo = max(0, 128 * j - 32)
                hi = min(L, 128 * (j + 1) + 32)
                Wj = hi - lo
                pex = []
                for h in range(H):
                    t = pex_p.tile([128, 256], bf16, tag="pex")
                    nc.scalar.activation(
                        t[:, 0:Wj], dist_sb[:, j, lo:hi], Exp,
                        scale=invsig_sb[:, j, h:h + 1],
                    )
                    pex.append(t)
                s01 = tree_p.tile([128, 256], bf16, tag="tr")
                s23 = tree_p.tile([128, 256], bf16, tag="tr")
                s45 = tree_p.tile([128, 256], bf16, tag="tr")
                s67 = tree_p.tile([128, 256], bf16, tag="tr")
                nc.gpsimd.tensor_tensor(s01[:, 0:Wj], pex[0][:, 0:Wj], pex[1][:, 0:Wj], add)
                nc.gpsimd.tensor_tensor(s23[:, 0:Wj], pex[2][:, 0:Wj], pex[3][:, 0:Wj], add)
                nc.vector.tensor_tensor(s45[:, 0:Wj], pex[4][:, 0:Wj], pex[5][:, 0:Wj], add)
                nc.vector.tensor_tensor(s67[:, 0:Wj], pex[6][:, 0:Wj], pex[7][:, 0:Wj], add)
                s0123 = tree_p.tile([128, 256], bf16, tag="tr2")
                s4567 = tree_p.tile([128, 256], bf16, tag="tr2")
                nc.vector.tensor_tensor(s0123[:, 0:Wj], s01[:, 0:Wj], s23[:, 0:Wj], add)
                nc.vector.tensor_tensor(s4567[:, 0:Wj], s45[:, 0:Wj], s67[:, 0:Wj], add)
                nc.vector.tensor_tensor(psT_sb[:, j, lo:hi], s0123[:, 0:Wj], s4567[:, 0:Wj], add)

            def prior_out_block(c):
                jlo = max(0, c - 1)
                jhi = min(NT, c + 2)
                nb = jhi - jlo
                Ws = 128 * nb
                pT = pmix_p.tile([128, 3, 128], bf16, tag="pmix")
                for jj, j in enumerate(range(jlo, jhi)):
                    nc.tensor.transpose(pT[:, jj, :],
                                        psT_sb[:, j, 128 * c:128 * (c + 1)],
                                        idb_sb[:])
                denp = col_p.tile([128, 1], f32, tag="col")
                nc.vector.tensor_reduce(denp[:], pT[:, 0:nb, :],
                                        axis=mybir.AxisListType.XYZW,
                                        op=add)
                rp = col_p.tile([128, 1], f32, tag="col")
                nc.vector.reciprocal(rp[:], denp[:])
                rows3 = prow_p.tile([128, 3, 128], bf16, tag="prow3")
                nc.vector.tensor_scalar_mul(rows3[:, 0:nb, :], pT[:, 0:nb, :], rp[:])
                nc.sync.dma_start(
                    pri_d[128 * c:128 * (c + 1), 128 * jlo:128 * jhi],
                    rows3[:, 0:nb, :],
                )

            def head_block(h):
                denh = col_p.tile([128, NT], f32, tag="denh")
                rch = col_p.tile([128, NT], f32, tag="denh")
                esbs = []
                for i in range(NT):
                    pre = 128 * i
                    W = pre + 128
                    psc = psc_p.tile([128, 1024], f32, tag="psc")
                    qsl = qt_sb[:, h, pre:pre + 128]
                    nch = (W + 511) // 512
                    for cc in range(nch):
                        e0 = 512 * cc
                        e1 = min(W, 512 * (cc + 1))
                        last = cc == nch - 1
                        nc.tensor.matmul(
                            psc[:, e0:e1], qsl, kt_sb[:, h, e0:e1],
                            start=True, stop=not last,
                            skip_group_check=True,
                        )
                    nc.tensor.matmul(psc[:, pre:W], idb_sb[:], mask_sb[:],
                                     start=False, stop=True,
                                     skip_group_check=True)
                    esb = esb_p.tile([128, 1024], bf16, tag="esb")
                    nc.scalar.activation(esb[:, 0:W], psc[:, 0:W], Exp,
                                         scale=SCALE, accum_out=denh[:, i:i + 1])
                    esbs.append(esb)
                nc.vector.reciprocal(rch[:], denh[:])
                vT = vts_p.tile([64, L], bf16, tag="vT")
                for i in range(NT):
                    pre = 128 * i
                    W = pre + 128
                    esb = esbs[i]
                    ssb = ssb_p.tile([128, 1024], bf16, tag="ssb")
                    nc.vector.tensor_scalar_mul(ssb[:, 0:W], esb[:, 0:W],
                                                rch[:, i:i + 1])
                    nc.gpsimd.dma_start(ser_d[h, pre:pre + 128, 0:W], ssb[:, 0:W])
                    at = pat_p.tile([128, 8, 128], bf16, tag="paT")
                    for j in range(i + 1):
                        nc.tensor.transpose(at[:, j, :],
                                            ssb[:, 128 * j:128 * (j + 1)],
                                            idb_sb[:])
                    nc.vector.tensor_copy(aT_sb[:, 0:i + 1, pre:pre + 128],
                                          at[:, 0:i + 1, :])
                for cch in range(2):
                    pv = pmix_p.tile([64, 512], f32, tag="pmix")
                    njv = 4 * (cch + 1)
                    for j in range(njv):
                        nc.tensor.matmul(
                            pv[:], v_sb[:, j, D * h:D * (h + 1)],
                            aT_sb[:, j, 512 * cch:512 * (cch + 1)],
                            start=(j == 0), stop=(j == njv - 1),
                        )
                    nc.vector.tensor_copy(vT[:, 512 * cch:512 * (cch + 1)], pv[:])
                prE = pmix_p.tile([128, 4, 64], bf16, tag="pmix")
                prO = pmix_p.tile([128, 4, 64], bf16, tag="pmix")
                for t in range(NT):
                    dst = prE if t % 2 == 0 else prO
                    nc.tensor.transpose(dst[:, t // 2, :],
                                        vT[:, 128 * t:128 * (t + 1)],
                                        idb_sb[0:64, 0:64])
                nc.vector.tensor_copy(vout_sb[:, 0:4, D * h:D * (h + 1)], prE[:])
                nc.vector.tensor_copy(vout_sb[:, 4:8, D * h:D * (h + 1)], prO[:])

            for h in range(H):
                load_block(h)
                prior_exp_block(h)
                if h >= 1:
                    prior_out_block(h - 1)
                head_block(h)
            prior_out_block(NT - 1)

            for t in range(NT):
                tsl = t // 2 if t % 2 == 0 else 4 + t // 2
                nc.sync.dma_start(vo_d[128 * t:128 * (t + 1), :],
                                  vout_sb[:, tsl, :])

    nc.finalize()
    return nc


_NC_CACHE = {}


def _get_nc():
    if "nc" not in _NC_CACHE:
        _NC_CACHE["nc"] = _build_graph()
    return _NC_CACHE["nc"]


def kernel(queries, keys, values, sigma):
    from concourse.bass_utils import run_bass_kernel_spmd

    assert queries.shape == (B, L, H, E)
    nc = _get_nc()

    idb = np.eye(128, dtype=np.float32).astype(BF16)
    maskb = np.triu(np.full((128, 128), NEG, dtype=np.float32), k=1).astype(BF16)
    ndist = -np.abs(
        np.arange(L, dtype=np.float32)[:, None] - np.arange(S, dtype=np.float32)[None, :]
    ).astype(np.float32)
    ndist_b = ndist.astype(BF16)

    in_maps = []
    for b in range(NCORES):
        in_maps.append({
            "qt": np.ascontiguousarray(queries[b].astype(BF16).transpose(1, 2, 0)),
            "kt": np.ascontiguousarray(keys[b].astype(BF16).transpose(1, 2, 0)),
            "v": np.ascontiguousarray(values[b].astype(BF16).reshape(S, H * D)),
            "sigt": np.ascontiguousarray(sigma[b].T.astype(np.float32)),
            "ndist": ndist_b,
            "idb": idb,
            "maskb": maskb,
        })

    trace = bool(int(os.environ.get("ANOM_TRACE", "0")))
    res = run_bass_kernel_spmd(nc, in_maps, core_ids=list(range(NCORES)), trace=trace)
    if trace and res.exec_time_ns is not None:
        print(f"HW exec time: {res.exec_time_ns} ns")
        if res.instructions_and_trace is not None:
            print("trace:", res.instructions_and_trace[1])

    Vout = np.zeros((B, L, H, D), dtype=np.float32)
    series = np.zeros((B, H, L, S), dtype=np.float32)
    prior = np.zeros((B, L, S), dtype=np.float32)
    for b in range(NCORES):
        r = res.results[b]
        Vout[b] = r["vout"].astype(np.float32).reshape(L, H, D)
        sb = r["series"].astype(np.float32)
        for i in range(L // 128):
            sb[:, 128 * i:128 * (i + 1), 128 * (i + 1):] = 0.0
        series[b] = sb
        pb = r["prior"].astype(np.float32)
        for c in range(L // 128):
            jlo, jhi = max(0, 128 * (c - 1)), min(S, 128 * (c + 2))
            prior[b, 128 * c:128 * (c + 1), :jlo] = 0.0
            prior[b, 128 * c:128 * (c + 1), jhi:] = 0.0
            prior[b, 128 * c:128 * (c + 1), jlo:jhi] = pb[128 * c:128 * (c + 1), jlo:jhi]
    return Vout, series, prior
